# revision 4
# baseline (speedup 1.0000x reference)
"""Trainium2 Bass kernel for causal multi-head attention (B=4,S=2048,D=1024,N=16,H=64).

Sharding: 8 cores = (batch b in 0..3) x (head-group g in 0..1). Each core gets
residual[b] and 8 heads' worth of W_Q/K/V/O, computes the partial output
sum_{n in group} attn_n @ W_O[n]  ->  [2048,1024]; host adds the two
head-group partials per batch. No collectives needed.

v2 design (vs baseline):
  - loads: SWDGE f32->f16 cast loads + XBAR DMA transposes build X^T/W^T
    (no PE transposes, no DVE evacs for loads)
  - projections: all N=512; V-proj produces all 8 heads per s-tile
  - QK^T: 2 heads row-tiled (64-contraction at tile_position rows 0/64),
    S^T for both heads land in one [128,1024] PSUM tile (2 banks)
  - exp: one batched ACT op over both heads' S^T (amortizes 352-cyc fixed)
  - PV: AO^T orientation: stationary V'=[V|ones(den)] (M=65/128), moving
    pts (N=512-o); denominator comes out as an extra AO^T row
  - normalize: DVE reciprocal_approx_fast on den row -> GPSIMD
    partition_broadcast -> fused DVE multiply-evac PSUM->aot (f16)
  - O-projection from aot (lhsT) as baseline, interleaved into pair 3
"""

import sys

sys.path.insert(0, "/opt/trn_rl_repo")

import numpy as np
import concourse.bass as bass
import concourse.bass_isa as bass_isa
import concourse.mybir as mybir
import concourse.tile as tile
from concourse import library_config
from concourse.bass_utils import run_bass_kernel_spmd

F32 = mybir.dt.float32
F16 = mybir.dt.float16
AF = mybir.ActivationFunctionType

S = 2048
D = 1024
NH = 8  # heads per core
H = 64
P = 128
ST = S // P  # 16
DT = D // P  # 8
NPAIR = NH // 2  # 4
SCALE = 1.0 / 8.0  # 1/sqrt(H)
COMPUTE_MAX_WAITS = 1
INTERLEAVE = 0  # 0=none (all proj upfront), 1=chain-contiguous pulls, 2=fine pull(2)

CTRL_INSTS = ("InstDrain", "InstNop", "InstEventSemaphoreOp", "InstSemaphoreOp")


def split_excess_waits(nc, max_waits=1, compute_max_waits=1):
    """This walrus build rejects >1 sync wait on CTRL-class instructions
    (Drain/NoOp). Move excess waits onto same-engine NOPs inserted
    immediately before. Compute instructions may keep compute_max_waits."""
    n_split = 0
    for bb in nc.main_func.blocks:
        insts = list(bb.instructions)
        out = []
        for ins in insts:
            si = ins.sync_info
            lim = max_waits if type(ins).__name__ in CTRL_INSTS else compute_max_waits
            if si is not None and si.on_wait and len(si.on_wait) > lim:
                waits = list(si.on_wait)
                while len(waits) > lim:
                    chunk, waits = waits[:1], waits[1:]
                    nop = nc.engines[ins.engine].nop(nofuse=True).ins
                    for b2 in nc.main_func.blocks:
                        if nop in b2.instructions:
                            b2.instructions.remove(nop)
                            break
                    if nop.sync_info is None:
                        nop.sync_info = mybir.SyncInfo(on_wait=[], on_update=[])
                    nop.sync_info.on_wait = chunk
                    out.append(nop)
                    n_split += 1
                si.on_wait = waits
            out.append(ins)
        bb.instructions[:] = out
    return n_split


def raise_dma_transpose_waits(nc):
    """DMA completions increment their DMAHW/DMASW semaphore +1 per DMA
    engine (16 total per op). Concurrent DMAs on the same lane interleave
    increments, so a consumer waiting >=16k can fire while DMA k is still in
    flight. Conservative fix: raise every wait on a lane that transposes
    touch to the cumulative total of ALL DMA increments emitted so far on
    that lane. All sync edges stay backward in emission order -> no cycles."""
    lane_totals = {}  # sem id -> running total of DMA increments
    tp_lanes = set()  # lanes that any InstDmaTransposeAnt updates
    # first pass: find transpose lanes
    for bb in nc.main_func.blocks:
        for ins in bb.instructions:
            if type(ins).__name__ == "InstDmaTransposeAnt" and ins.sync_info:
                for u in ins.sync_info.on_update:
                    tp_lanes.add(u.id)
    n_raised = 0
    for bb in nc.main_func.blocks:
        for ins in bb.instructions:
            si = ins.sync_info
            if si is None:
                continue
            tn = type(ins).__name__
            if si.on_wait:
                for w in si.on_wait:
                    if w.id in tp_lanes:
                        tot = lane_totals.get(w.id, 0)
                        if w.wait_value < tot:
                            w.wait_value = tot
                            n_raised += 1
            if ("DMA" in tn or "Dma" in tn) and si.on_update:
                for u in si.on_update:
                    lane_totals[u.id] = lane_totals.get(u.id, 0) + u.update_value
    return n_raised


def emit(nc, tc, x, wq_d, wk_d, wv_d, wo_d, out_d, dbg=None):
    with (
        tc.tile_pool(name="persist", bufs=1) as persist,
        tc.tile_pool(name="psM", bufs=1, space="PSUM") as psM,
    ):
        xt = persist.tile([P, DT, S], F16)  # X^T: xt[p,k,s] = X[s, k*128+p]
        wqt = persist.tile([P, DT, 512], F16)  # W^T: wqt[p,k,c] = Wq[c, k*128+p]
        wkt = persist.tile([P, DT, 512], F16)
        wvt = persist.tile([P, DT, 512], F16)
        wo_sb = persist.tile([P, 4, D], F16)  # wo_sb[p,a,d] = Wo[a*128+p, d]
        qt = persist.tile([P, NPAIR * S], F16)  # Q^T per pair (2 heads/128 rows)
        kt = persist.tile([P, NPAIR * S], F16)
        # vv[p, i, t, :]: [V_A(64) | ones(1) | zeros(63) | V_B(64)] for sk-tile i
        vv = persist.tile([P, ST, NPAIR, 192], F16)
        aot = persist.tile([P, NPAIR * S], F16)  # normalized AO^T

        nc.gpsimd.load_library(library_config.attn)

        # dz/dzB: zero tiles except one row carrying 1/den per (t,G); the
        # partition_all_reduce of the whole tile then equals a broadcast of
        # that row (HW partition_broadcast can only source/write partition-0
        # based ranges, so it can't serve either head here).
        dz = persist.tile([P, 512], F32)
        dzB = persist.tile([P, 512], F32)

        # ---- Phase 0: loads.  SWDGE cast loads (f32->f16) + XBAR transposes.
        stage_cm = tc.tile_pool(name="stage", bufs=1)
        stage = stage_cm.__enter__()
        if True:
            # weights first (q/k pair 0 unblock PE), X groups after
            wsts = []
            for wd in (wq_d, wk_d, wv_d):
                w16 = stage.tile([P, 4, D], F16, tag="w16", bufs=2)
                nc.gpsimd.dma_start(
                    out=w16, in_=wd[:, :].rearrange("(a p) d -> p a d", p=P)
                )
                wsts.append(w16)
            nc.gpsimd.dma_start(
                out=wo_sb, in_=wo_d[:, :].rearrange("(a p) d -> p a d", p=P)
            )
            # vv constant columns
            nc.gpsimd.memset(vv[:, :, :, 64:65], 1.0)
            nc.gpsimd.memset(vv[:, :, :, 65:128], 0.0)
            nc.gpsimd.memset(dz, 0.0)
            nc.gpsimd.memset(dzB, 0.0)

            xs_by_g = {}
            for g in range(4):
                xg = stage.tile([P, 4, D], F16, tag="xs", bufs=2)
                nc.gpsimd.dma_start(
                    out=xg,
                    in_=x[g * 512 : (g + 1) * 512, :].rearrange(
                        "(a p) d -> p a d", p=P
                    ),
                )
                xs_by_g[g] = xg

            # transposes (SP HWDGE queue; 14ns/16x128 tile)
            for wi, (w16, wt) in enumerate(zip(wsts, (wqt, wkt, wvt))):
                for a in range(4):
                    nc.sync.dma_start(
                        out=wt[:, :, a * P : (a + 1) * P],
                        in_=w16[:, a, :],
                        transpose=True,
                    )
            for g in range(4):
                xg = xs_by_g.pop(g)
                for a in range(4):
                    i = 4 * g + a
                    nc.sync.dma_start(
                        out=xt[:, :, i * P : (i + 1) * P],
                        in_=xg[:, a, :],
                        transpose=True,
                    )
        stage_cm.__exit__(None, None, None)

        if True:
            # ---- projection chain generators (yield after each matmul)
            def gen_qk(t, c, src, dst):
                pq = psM.tile([P, 512], F32, tag="pq", bufs=2)
                for k in range(DT):
                    nc.tensor.matmul(
                        pq,
                        lhsT=src[:, k, t * P : (t + 1) * P],
                        rhs=xt[:, k, c * 512 : (c + 1) * 512],
                        start=(k == 0),
                        stop=(k == DT - 1),
                    )
                    yield
                nc.vector.tensor_copy(
                    dst[:, t * S + c * 512 : t * S + (c + 1) * 512], pq
                )

            def gen_v(i):
                pv = psM.tile([P, 512], F32, tag="pq", bufs=2)
                for k in range(DT):
                    nc.tensor.matmul(
                        pv,
                        lhsT=xt[:, k, i * P : (i + 1) * P],
                        rhs=wvt[:, k, :],
                        start=(k == 0),
                        stop=(k == DT - 1),
                    )
                    yield
                pvv = pv.rearrange("p (t hh) -> p t hh", t=NPAIR)
                nc.vector.tensor_copy(vv[:, i, :, 0:64], pvv[:, :, 0:64])
                nc.vector.tensor_copy(vv[:, i, :, 128:192], pvv[:, :, 64:128])

            def drain(g):
                for _ in g:
                    pass

            # phase 1 (PE prologue): pair-0 q/k + v chains 0..3
            for c in range(4):
                drain(gen_qk(0, c, wqt, qt))
                drain(gen_qk(0, c, wkt, kt))
            if INTERLEAVE == 0:
                for i in range(ST):
                    drain(gen_v(i))
                for t in range(1, NPAIR):
                    for c in range(4):
                        drain(gen_qk(t, c, wqt, qt))
                        drain(gen_qk(t, c, wkt, kt))
                v_done = ST
                vpend = []
                qkpend = {}
            else:
                for i in range(4):
                    drain(gen_v(i))
                v_done = 4
                vpend = [(i, gen_v(i)) for i in range(4, ST)]
                qkpend = {
                    t: [g for c in range(4) for g in (gen_qk(t, c, wqt, qt), gen_qk(t, c, wkt, kt))]
                    for t in range(1, NPAIR)
                }

            # ---- Phase 2: attention per pair / sq-group / sk-tile
            for t in range(NPAIR):
                gens = list(qkpend.get(t + 1, []))
                gens.reverse()

                def pull(n):
                    for _ in range(n):
                        while gens:
                            try:
                                next(gens[-1])
                                break
                            except StopIteration:
                                gens.pop()
                        if not gens:
                            return

                def pull_chain():
                    # drain exactly one full generator, keeping its matmul
                    # accumulation group contiguous in the PE stream
                    while gens:
                        try:
                            next(gens[-1])
                        except StopIteration:
                            gens.pop()
                            return

                def ensure_v(upto):
                    nonlocal v_done
                    while v_done <= upto and vpend:
                        _, g = vpend.pop(0)
                        drain(g)
                        v_done += 1

                for G in range(4):
                    njs = 4 * G + 4
                    if t == 0:
                        ensure_v(njs - 1)
                    sts = [None] * njs
                    pts_by_j = [None] * njs
                    po = psM.tile([P, 1024], F32, tag="po", bufs=1)

                    def emit_st(j):
                        o = max(0, (j - 4 * G) * P)
                        st = psM.tile([P, 1024], F32, tag="st", bufs=2)
                        # head A rows 0:64, head B rows 64:128 -- row-tiled pair
                        nc.tensor.matmul(
                            st[:, o:512],
                            lhsT=kt[0:64, t * S + j * P : t * S + (j + 1) * P],
                            rhs=qt[0:64, t * S + G * 512 + o : t * S + (G + 1) * 512],
                            start=True,
                            stop=True,
                        )
                        nc.tensor.matmul(
                            st[:, 512 + o : 1024],
                            lhsT=kt[64:128, t * S + j * P : t * S + (j + 1) * P],
                            rhs=qt[64:128, t * S + G * 512 + o : t * S + (G + 1) * 512],
                            start=True,
                            stop=True,
                        )
                        sts[j] = (st, o)

                    def emit_exp(j):
                        st, o = sts[j]
                        pts = persist.tile([P, 1024], F16, tag="pts", bufs=5, name="pts")
                        if o == 0:
                            nc.scalar.activation(pts, st, AF.Exp, scale=SCALE)
                        else:
                            nc.scalar.activation(
                                pts[:, o:512], st[:, o:512], AF.Exp, scale=SCALE
                            )
                            nc.scalar.activation(
                                pts[:, 512 + o : 1024],
                                st[:, 512 + o : 1024],
                                AF.Exp,
                                scale=SCALE,
                            )
                        if j >= 4 * G:
                            for base in (o, 512 + o):
                                nc.gpsimd.affine_select(
                                    out=pts[:, base : base + P],
                                    in_=pts[:, base : base + P],
                                    compare_op=mybir.AluOpType.is_ge,
                                    fill=0.0,
                                    base=0,
                                    pattern=[[1, P]],
                                    channel_multiplier=-1,
                                )
                        pts_by_j[j] = (pts, o)

                    emit_st(0)
                    emit_exp(0)
                    for j in range(njs):
                        if j + 1 < njs:
                            emit_st(j + 1)
                            emit_exp(j + 1)
                        if INTERLEAVE == 2:
                            pull(2)
                        elif INTERLEAVE == 1:
                            pull_chain()
                        pts, o = pts_by_j[j]
                        pts_by_j[j] = None
                        sts[j] = None
                        # PV head A: AO^T rows 0:64 + den row 64
                        nc.tensor.matmul(
                            po[0:65, o:512],
                            lhsT=vv[:, j, t, 0:65],
                            rhs=pts[:, o:512],
                            start=(j == 0),
                            stop=(j == njs - 1),
                            skip_group_check=True,
                        )
                        # PV head B: den row 0, AO^T rows 64:128
                        nc.tensor.matmul(
                            po[0:128, 512 + o : 1024],
                            lhsT=vv[:, j, t, 64:192],
                            rhs=pts[:, 512 + o : 1024],
                            start=(j == 0),
                            stop=(j == njs - 1),
                            skip_group_check=True,
                        )

                    # normalize + evac to aot.  Head A's den sits at PSUM
                    # partition 64: plain DVE copy into dz row 64 (custom DVE
                    # ops silently no-op at base partition 64 on HW), then
                    # partition_all_reduce of the zero-elsewhere tile (= a
                    # broadcast of row 64), then recip at base partition 0.
                    nc.vector.tensor_copy(dz[64:65, :], po[64:65, 0:512])
                    rba = persist.tile([P, 512], F32, tag="rba", bufs=2, name="rba")
                    nc.gpsimd.partition_all_reduce(
                        rba, dz, 128, bass_isa.ReduceOp.add
                    )
                    rb2 = persist.tile([P, 512], F32, tag="rb2", bufs=2, name="rb2")
                    nc.vector.reciprocal_approx_fast(
                        out=rb2[0:64, :], in_=rba[0:64, :]
                    )
                    # Head B: den at PSUM partition 0 -> recip into dzB row 0,
                    # all_reduce-broadcast
                    nc.vector.reciprocal_approx_fast(
                        out=dzB[0:1, :], in_=po[0:1, 512:1024]
                    )
                    rb = persist.tile([P, 512], F32, tag="rb", bufs=2, name="rb")
                    nc.gpsimd.partition_all_reduce(
                        rb, dzB, 128, bass_isa.ReduceOp.add
                    )
                    acols = slice(t * S + G * 512, t * S + (G + 1) * 512)
                    nc.vector.tensor_tensor(
                        out=aot[0:64, acols],
                        in0=po[0:64, 0:512],
                        in1=rb2[0:64, :],
                        op=mybir.AluOpType.mult,
                    )
                    nc.vector.tensor_tensor(
                        out=aot[64:128, acols],
                        in0=po[64:128, 512:1024],
                        in1=rb[64:128, :],
                        op=mybir.AluOpType.mult,
                    )

                    if t == NPAIR - 1:
                        # O-projection for this sq group (interleaved into the
                        # last pair's ACT-bound stream)
                        for ii in range(4):
                            i = 4 * G + ii
                            osb = persist.tile([P, D], F32, tag="osb", bufs=3, name="osb")
                            for c in range(2):
                                oo = psM.tile([P, 512], F32, tag="pq", bufs=2)
                                for tp in range(NPAIR):
                                    nc.tensor.matmul(
                                        oo,
                                        lhsT=aot[:, tp * S + i * P : tp * S + (i + 1) * P],
                                        rhs=wo_sb[:, tp, c * 512 : (c + 1) * 512],
                                        start=(tp == 0),
                                        stop=(tp == NPAIR - 1),
                                    )
                                nc.vector.tensor_copy(
                                    osb[:, c * 512 : (c + 1) * 512], oo
                                )
                            nc.sync.dma_start(
                                out=out_d[i * P : (i + 1) * P, :], in_=osb
                            )
                pull(10**6)

        if dbg is not None:
            nc.gpsimd.dma_start(out=dbg["xt"][:, :], in_=xt.rearrange("p k s -> p (k s)"))
            nc.gpsimd.dma_start(out=dbg["wqt"][:, :], in_=wqt.rearrange("p k c -> p (k c)"))
            nc.gpsimd.dma_start(out=dbg["wkt"][:, :], in_=wkt.rearrange("p k c -> p (k c)"))
            nc.gpsimd.dma_start(out=dbg["qt"][:, :], in_=qt)
            nc.gpsimd.dma_start(out=dbg["kt"][:, :], in_=kt)
            nc.gpsimd.dma_start(out=dbg["vv"][:, :], in_=vv.rearrange("p i t e -> p (i t e)"))
            nc.gpsimd.dma_start(out=dbg["aot"][:, :], in_=aot)


def build_nc(debug=False):
    nc = bass.Bass()
    x = nc.dram_tensor("x", [S, D], F32, kind="ExternalInput")
    wq_d = nc.dram_tensor("wq", [NH * H, D], F32, kind="ExternalInput")
    wk_d = nc.dram_tensor("wk", [NH * H, D], F32, kind="ExternalInput")
    wv_d = nc.dram_tensor("wv", [NH * H, D], F32, kind="ExternalInput")
    wo_d = nc.dram_tensor("wo", [NH * H, D], F32, kind="ExternalInput")
    out_d = nc.dram_tensor("out", [S, D], F32, kind="ExternalOutput")
    dbg = None
    if debug:
        dbg = {
            "xt": nc.dram_tensor("dbg_xt", [P, DT * S], F32, kind="ExternalOutput"),
            "wqt": nc.dram_tensor("dbg_wqt", [P, DT * 512], F32, kind="ExternalOutput"),
            "wkt": nc.dram_tensor("dbg_wkt", [P, DT * 512], F32, kind="ExternalOutput"),
            "qt": nc.dram_tensor("dbg_qt", [P, NPAIR * S], F32, kind="ExternalOutput"),
            "kt": nc.dram_tensor("dbg_kt", [P, NPAIR * S], F32, kind="ExternalOutput"),
            "vv": nc.dram_tensor("dbg_vv", [P, ST * NPAIR * 192], F32, kind="ExternalOutput"),
            "aot": nc.dram_tensor("dbg_aot", [P, NPAIR * S], F32, kind="ExternalOutput"),
        }
    with tile.TileContext(nc) as tc:
        emit(nc, tc, x, wq_d, wk_d, wv_d, wo_d, out_d, dbg=dbg)
    raise_dma_transpose_waits(nc)
    split_excess_waits(nc, compute_max_waits=COMPUTE_MAX_WAITS)
    from concourse import library_overlay

    library_overlay.lower_extended_insts(nc)
    return nc


_cache = {}


def _get_runner():
    """Persistent jitted 8-core runner (mirrors bass2jax.run_bass_via_pjrt's
    multi-core path, but reusable across calls so we can time executions)."""
    if "runner" in _cache:
        return _cache["runner"]
    import jax
    from jax.experimental.shard_map import shard_map
    from jax.sharding import Mesh, PartitionSpec
    from concourse import bass2jax

    bass2jax.install_neuronx_cc_hook()
    if "nc" not in _cache:
        _cache["nc"] = build_nc()
    nc = _cache["nc"]

    partition_name = nc.partition_id_tensor.name if nc.partition_id_tensor else None
    in_names, out_names, out_avals = [], [], []
    for alloc in nc.m.functions[0].allocations:
        if not isinstance(alloc, mybir.MemoryLocationSet):
            continue
        name = alloc.memorylocations[0].name
        if alloc.kind == "ExternalInput":
            if name != partition_name:
                in_names.append(name)
        elif alloc.kind == "ExternalOutput":
            out_names.append(name)
            out_avals.append(
                jax.core.ShapedArray(tuple(alloc.tensor_shape), mybir.dt.np(alloc.dtype))
            )
    n_params, n_outs = len(in_names), len(out_names)
    all_names = list(in_names) + list(out_names)
    if partition_name is not None:
        all_names.append(partition_name)
    all_names = tuple(all_names)

    def _body(*args):
        operands = list(args)
        if partition_name is not None:
            operands.append(bass2jax.partition_id_tensor())
        outs = bass2jax._bass_exec_p.bind(
            *operands,
            out_avals=tuple(out_avals),
            in_names=all_names,
            out_names=tuple(out_names),
            lowering_input_output_aliases=(),
            sim_require_finite=True,
            sim_require_nnan=True,
            nc=nc,
        )
        return tuple(outs)

    devices = jax.devices()[:8]
    mesh = Mesh(np.asarray(devices), ("core",))
    in_specs = (PartitionSpec("core"),) * (n_params + n_outs)
    out_specs = (PartitionSpec("core"),) * n_outs
    donate = tuple(range(n_params, n_params + n_outs))
    sharded = jax.jit(
        shard_map(_body, mesh=mesh, in_specs=in_specs, out_specs=out_specs, check_rep=False),
        donate_argnums=donate,
        keep_unused=True,
    )
    _cache["runner"] = (sharded, in_names, out_names, out_avals, mesh)
    return _cache["runner"]


def run_on_cores(in_maps):
    """Run the kernel on 8 cores; returns list of per-core output dicts."""
    sharded, in_names, out_names, out_avals, mesh = _get_runner()
    concat_in = [
        np.concatenate([np.asarray(in_maps[c][name]) for c in range(8)], axis=0)
        for name in in_names
    ]
    concat_zeros = [
        np.zeros((8 * a.shape[0], *a.shape[1:]), a.dtype) for a in out_avals
    ]
    out_arrs = sharded(*concat_in, *concat_zeros)
    return [
        {
            name: np.asarray(out_arrs[i]).reshape(8, *out_avals[i].shape)[c]
            for i, name in enumerate(out_names)
        }
        for c in range(8)
    ]


def make_in_maps(residual, W_Q, W_K, W_V, W_O):
    in_maps = []
    for core in range(8):
        b, g = core // 2, core % 2
        sl = slice(8 * g, 8 * (g + 1))
        in_maps.append(
            {
                "x": np.ascontiguousarray(residual[b], dtype=np.float32),
                "wq": np.ascontiguousarray(W_Q[sl].reshape(NH * H, D), dtype=np.float32),
                "wk": np.ascontiguousarray(W_K[sl].reshape(NH * H, D), dtype=np.float32),
                "wv": np.ascontiguousarray(W_V[sl].reshape(NH * H, D), dtype=np.float32),
                "wo": np.ascontiguousarray(W_O[sl].reshape(NH * H, D), dtype=np.float32),
            }
        )
    return in_maps


def kernel(residual, W_Q, W_K, W_V, W_O):
    residual = np.asarray(residual)
    in_maps = make_in_maps(residual, W_Q, W_K, W_V, W_O)
    results = run_on_cores(in_maps)
    B = residual.shape[0]
    out = np.zeros((B, S, D), np.float32)
    for core in range(8):
        b = core // 2
        out[b] += results[core]["out"]
    return out


if __name__ == "__main__":
    rng = np.random.default_rng(0)
    residual = rng.standard_normal((4, S, D)).astype(np.float32)
    W = [0.02 * rng.standard_normal((16, H, D)).astype(np.float32) for _ in range(4)]
    out = kernel(residual, *W)
    print("kernel ran, out shape", out.shape, "finite:", np.isfinite(out).all())
